# revision 1
# baseline (speedup 1.0000x reference)
"""Causal depthwise conv1d (K=3) + pointwise 1x1 conv for Trainium2.

Full-input contract: kernel(**inputs) takes the complete (unsharded) numpy
inputs and returns the complete output. Internally the work is sharded over
8 NeuronCores: core c handles batch b = c//2 and sequence half c%2
(L_chunk = 2048), with a (K-1)=2 column halo taken from the previous
sequence chunk (zeros at the causal left edge). The small conv weights are
replicated on every core.

Per-core layout is channel-major: x is pre-transposed on the host to
(D, 2 + L_chunk) so the depthwise conv runs as per-partition
scalar*tensor ops on DVE/ACT and the pointwise conv becomes a plain
K-contraction matmul on the PE array (fp32r at full rate).
"""

import sys

if "/opt/trn_rl_repo" not in sys.path:
    sys.path.insert(0, "/opt/trn_rl_repo")

import numpy as np

import concourse.bass as bass
import concourse.tile as tile
from concourse import bacc, mybir
from concourse.bass_utils import run_bass_kernel_spmd

P = 128          # SBUF partitions
B, L, D = 4, 4096, 1024
KSZ = 3          # depthwise kernel taps
HALO = KSZ - 1
NCORES = 8
LC = (B * L) // NCORES   # 2048 sequence positions per core
# l-tile schedule: small first tiles shrink the serial prologue (the PE can
# start as soon as the first 256 columns of y exist); 512 = one PSUM bank
LTS = [512, 512, 512, 512]
assert sum(LTS) == LC
DC = D // P              # 8 channel chunks (contraction)
EC = D // P              # 8 output-channel chunks

# Precision mode for the PE operands (w_pw slabs, y, and the x DMA):
#   "bf16"  - halves w/x DMA traffic, PE rate 1; rel err ~2.5e-3
#   "fp32r" - full fp32 I/O, PE still rate 1 at N>=256; rel err ~1.3e-4
PRECISION = "bf16"
MM_DT = mybir.dt.bfloat16 if PRECISION == "bf16" else mybir.dt.float32r

# REPEAT > 1 wraps the whole kernel body in a hardware For_i loop; used by
# bench.py to measure real per-iteration HW time via wall-clock deltas.
REPEAT = 1

# Diagnostic ablations for bench experiments (None for the real kernel):
#   "nodw"    - skip depthwise conv, feed raw x chunks to the PE
#   "nostore" - skip all but one output store
#   "noload"  - skip x input DMAs (PE consumes uninitialized SBUF)
EXP = None

_CACHED_NC = None


def _build_nc():
    nc = bacc.Bacc("TRN2", target_bir_lowering=False, debug=False,
                   num_devices=NCORES)
    f32 = mybir.dt.float32

    # x rides the matmul precision: bf16 halves input DMA traffic
    X_DT = MM_DT if PRECISION == "bf16" else f32
    xt = nc.dram_tensor("xt", [D, HALO + LC], X_DT, kind="ExternalInput").ap()
    # weights pre-swizzled on the host: wt[ec, p, dc*P+j] = w_pw[ec*P+j, dc*P+p]
    # so each e-slice DMA is a (128, DC*P) slab with contiguous 2 KB lines
    wt = nc.dram_tensor("wt", [EC, P, DC * P], MM_DT, kind="ExternalInput").ap()
    # per-channel params, columns: w_dw[0..2], b_dw, b_pw
    pp = nc.dram_tensor("pp", [D, 5], f32, kind="ExternalInput").ap()
    ot = nc.dram_tensor("ot", [D, LC], f32, kind="ExternalOutput").ap()

    xt_r = xt.rearrange("(o p) c -> p o c", p=P)   # [128, DC, HALO+LC]
    pp_r = pp.rearrange("(o p) c -> p o c", p=P)   # [128, DC, 5]
    ot_r = ot.rearrange("(o p) l -> p o l", p=P)   # [128, EC, LC]

    lt_off = [0]
    for n in LTS:
        lt_off.append(lt_off[-1] + n)
    max_lt = max(LTS)

    from contextlib import nullcontext
    with tile.TileContext(nc) as tc:
        loop_ctx = tc.For_i(0, REPEAT, 1) if REPEAT > 1 else nullcontext()
        with (
            loop_ctx,
            tc.tile_pool(name="wpool", bufs=1) as wpool,
            tc.tile_pool(name="ppool", bufs=1) as ppool,
            tc.tile_pool(name="xpool", bufs=3) as xpool,
            tc.tile_pool(name="tpool", bufs=4) as tpool,
            tc.tile_pool(name="ypool", bufs=26) as ypool,
            tc.tile_pool(name="opool", bufs=6) as opool,
            tc.tile_pool(name="psum", bufs=8, space="PSUM") as psum_pool,
        ):
            # per-channel params first (tiny), then the first e-slice of the
            # pointwise weights so the PE's first accumulation group isn't
            # gated on the full 4 MB weight load
            p_sb = ppool.tile([P, DC, 5], f32)
            # weight slabs in three tiles: ec0 alone (gates the first PE
            # group), then ec1-3 and ec4-7 batched (fewer DMA launches)
            w_sb0 = wpool.tile([P, 1, DC * P], MM_DT, name="w_sb0")
            w_sb13 = wpool.tile([P, 3, DC * P], MM_DT, name="w_sb13")
            w_sb47 = wpool.tile([P, 4, DC * P], MM_DT, name="w_sb47")

            def w_ap(ec, dc):
                if ec == 0:
                    return w_sb0[:, 0, dc * P:(dc + 1) * P]
                if ec < 4:
                    return w_sb13[:, ec - 1, dc * P:(dc + 1) * P]
                return w_sb47[:, ec - 4, dc * P:(dc + 1) * P]

            def x_load(lt):
                """one batched DMA for all channel chunks of l-tile lt"""
                n = LTS[lt]
                o = lt_off[lt]
                xs = xpool.tile([P, DC, max_lt + HALO], X_DT, tag="x", name="xs")[:, :, :n + HALO]
                if EXP == "noload":
                    # tiny touch so the tile scheduler sees a write
                    nc.sync.dma_start(xs[:, :, 0:HALO], xt_r[:, :, o:o + HALO])
                else:
                    nc.sync.dma_start(xs[:], xt_r[:, :, o:o + n + HALO])
                return xs

            def dw_conv(lt, xs):
                """depthwise conv for all channel chunks of l-tile lt"""
                n = LTS[lt]
                if EXP == "nodw":
                    return [xs[:, dc, 0:n] for dc in range(DC)]
                ys = []
                for dc in range(DC):
                    x_t = xs[:, dc, :]
                    y_t = ypool.tile([P, max_lt], MM_DT, tag="y", name="y_t")[:, :n]
                    # y = w0*x[l-2] + b_dw + w1*x[l-1] + w2*x[l]; first tap on
                    # ACT for even chunks (engine balance), DVE 2x-mode
                    # tensor_scalar for odd; accumulating taps are DVE STT;
                    # the last write rounds to fp32r for the PE
                    t_t = tpool.tile([P, max_lt], f32, tag="t", name="t_t")[:, :n]
                    if dc % 2 == 0:
                        # taps 0+1 on ACT/DVE
                        nc.scalar.activation(
                            t_t[:], x_t[:, 0:n],
                            mybir.ActivationFunctionType.Identity,
                            bias=p_sb[:, dc, 3:4], scale=p_sb[:, dc, 0:1])
                        nc.vector.scalar_tensor_tensor(
                            t_t[:], x_t[:, 1:1 + n], p_sb[:, dc, 1:2], t_t[:],
                            op0=mybir.AluOpType.mult, op1=mybir.AluOpType.add)
                    else:
                        # tap 0 via DVE 2x-mode tensor_scalar, tap 1 DVE STT
                        nc.vector.tensor_scalar(
                            t_t[:], x_t[:, 0:n],
                            p_sb[:, dc, 0:1], p_sb[:, dc, 3:4],
                            op0=mybir.AluOpType.mult, op1=mybir.AluOpType.add)
                        nc.vector.scalar_tensor_tensor(
                            t_t[:], x_t[:, 1:1 + n], p_sb[:, dc, 1:2], t_t[:],
                            op0=mybir.AluOpType.mult, op1=mybir.AluOpType.add)
                    # final tap always on DVE: writes y rounded to fp32r
                    nc.vector.scalar_tensor_tensor(
                        y_t[:], x_t[:, 2:2 + n], p_sb[:, dc, 2:3], t_t[:],
                        op0=mybir.AluOpType.mult, op1=mybir.AluOpType.add)
                    ys.append(y_t)
                return ys

            def pointwise(lt, ys, ecs=range(EC)):
                """out[:, lt] = w_pw @ y[:, lt] + b_pw for e-chunks `ecs`"""
                n = LTS[lt]
                o = lt_off[lt]
                for ec in ecs:
                    acc = psum_pool.tile([P, max_lt], f32, tag="acc", name="acc")[:, :n]
                    for dc in range(DC):
                        nc.tensor.matmul(
                            acc[:],
                            lhsT=w_ap(ec, dc),
                            rhs=ys[dc][:],
                            start=(dc == 0), stop=(dc == DC - 1))
                    o_t = opool.tile([P, max_lt], f32, tag="o", name="o_t")[:, :n]
                    nc.scalar.activation(
                        o_t[:], acc[:],
                        mybir.ActivationFunctionType.Identity,
                        bias=p_sb[:, ec, 4:5], scale=1.0)
                    # stores ride the idle GpSimd SWDGE queue; the final
                    # tile alternates with SP (idle by then) to cut the tail
                    if EXP == "nostore":
                        if lt == len(LTS) - 1 and ec == EC - 1:
                            nc.sync.dma_start(ot_r[:, ec, o:o + n], o_t[:])
                    elif lt >= len(LTS) - 2 and ec % 2 == 0:
                        nc.sync.dma_start(ot_r[:, ec, o:o + n], o_t[:])
                    else:
                        nc.gpsimd.dma_start(ot_r[:, ec, o:o + n], o_t[:])

            # SP DMA FIFO order, hand-scheduled: x(lt0) gates the PE start;
            # weight slabs stream in behind the early x tiles. The first two
            # l-tiles run their e-chunk groups in two phases (ec0-3 then
            # ec4-7) so the PE only needs half the weights early. dw_conv for
            # tile lt+1 is always emitted before the pointwise groups of lt.
            nc.sync.dma_start(p_sb[:], pp_r[:])
            xs0 = x_load(0)
            nc.sync.dma_start(w_sb0[:], wt[0:1].rearrange("e p f -> p e f"))
            ys0 = dw_conv(0, xs0)
            nc.sync.dma_start(w_sb13[:], wt[1:4].rearrange("e p f -> p e f"))
            xs1 = x_load(1)
            ys1 = dw_conv(1, xs1)
            pointwise(0, ys0, range(0, 4))
            nc.sync.dma_start(w_sb47[:], wt[4:8].rearrange("e p f -> p e f"))
            xs2 = x_load(2)
            ys2 = dw_conv(2, xs2)
            pointwise(1, ys1, range(0, 4))
            pointwise(0, ys0, range(4, EC))
            pointwise(1, ys1, range(4, EC))
            ys_prev = ys2
            for lt in range(3, len(LTS) + 1):
                if lt < len(LTS):
                    xs = x_load(lt)
                    ys = dw_conv(lt, xs)
                else:
                    ys = None
                pointwise(lt - 1, ys_prev)
                ys_prev = ys

    nc.compile()  # bacc: legalizes multi-sem waits for TRN2 codegen
    return nc


def _shard_inputs(x, w_dw, b_dw, w_pw, b_pw):
    import ml_dtypes
    w_np = ml_dtypes.bfloat16 if PRECISION == "bf16" else np.float32
    x_np = ml_dtypes.bfloat16 if PRECISION == "bf16" else np.float32
    # wt[ec, p, dc*128+j] = w_pw[ec*128+j, dc*128+p]
    wt = np.ascontiguousarray(
        w_pw.reshape(EC, P, DC, P).transpose(0, 3, 2, 1).reshape(EC, P, DC * P)
    ).astype(w_np)
    pp = np.ascontiguousarray(
        np.stack([w_dw[:, 0], w_dw[:, 1], w_dw[:, 2], b_dw, b_pw], axis=1),
        dtype=np.float32)                                        # (D, 5)
    in_maps = []
    for c in range(NCORES):
        b, half = divmod(c, 2)
        l0 = half * LC
        xt = np.zeros((D, HALO + LC), dtype=x_np)
        lo = max(l0 - HALO, 0)
        xt[:, HALO - (l0 - lo):] = x[b, lo:l0 + LC, :].T
        in_maps.append({"xt": xt, "wt": wt, "pp": pp})
    return in_maps


def kernel(x, w_dw, b_dw, w_pw, b_pw):
    assert x.shape == (B, L, D) and w_dw.shape == (D, KSZ)
    global _CACHED_NC
    if _CACHED_NC is None:
        _CACHED_NC = _build_nc()
    in_maps = _shard_inputs(np.asarray(x, dtype=np.float32),
                            np.asarray(w_dw), np.asarray(b_dw),
                            np.asarray(w_pw), np.asarray(b_pw))
    results = run_bass_kernel_spmd(
        _CACHED_NC, in_maps, list(range(NCORES))).results
    out = np.empty((B, L, D), dtype=np.float32)
    for c in range(NCORES):
        b, half = divmod(c, 2)
        l0 = half * LC
        out[b, l0:l0 + LC, :] = results[c]["ot"].T
    return out

